# revision 1
# baseline (speedup 1.0000x reference)
"""Trainium2 Bass kernel for a 2-layer tanh DeepRNN.

Problem: inputs [64, 1024, 256] fp32, two stacked RNN layers (H=512):
    h0_t = tanh(x_t @ W_xh0 + h0_{t-1} @ W_hh0 + b_h0)
    h1_t = tanh(h0_t @ W_xh1 + h1_{t-1} @ W_hh1 + b_h1)
Output: h1 sequence [64, 1024, 512] fp32.

Strategy (per spec sharding hint): data-parallel over batch, 8 cores x
B_local=8, weights replicated. Inside each core everything runs in the
"transposed" domain: the recurrent state lives as hT [128, 4hc*8b] tiles so
the per-step matmuls are W-stationary (bf16 fast-weight-load) with the tiny
batch as the moving operand, and tanh runs on all 128 ACT lanes. The input
projections (x @ W_xh0, h0 @ W_xh1) are batched 16 timesteps at a time
directly into the same PSUM bank the recurrent matmuls then accumulate into;
biases enter via rank-1 (ones x b) matmuls. bf16 everywhere with fp32 PSUM
accumulation: measured absmax error vs the fp32 reference ~9e-4.
"""

import sys

import numpy as np

sys.path.insert(0, "/opt/trn_rl_repo")

import ml_dtypes  # noqa: E402

import concourse.bacc as bacc  # noqa: E402
import concourse.tile as tile  # noqa: E402
from concourse import mybir  # noqa: E402
from concourse.bass_utils import run_bass_kernel_spmd  # noqa: E402

F32 = mybir.dt.float32
BF16 = mybir.dt.bfloat16
Tanh = mybir.ActivationFunctionType.Tanh

B_FULL, T, I, H = 64, 1024, 256, 512
NCORES = 8
B = B_FULL // NCORES  # 8 rows per core
CH = 16               # timesteps per chunk (16*32 cols = one PSUM bank)
NCH = T // CH


def _mm(nc, out, lhsT, rhs, start, stop):
    nc.tensor.matmul(out, lhsT, rhs, start=start, stop=stop, skip_group_check=True)


def build_nc(nch=NCH, reps=1):
    nc = bacc.Bacc("TRN2", target_bir_lowering=False, debug=False)

    x_d = nc.dram_tensor("x", [B, T, I], F32, kind="ExternalInput")
    wxh0_d = nc.dram_tensor("W_xh0", [I, H], F32, kind="ExternalInput")
    whh0_d = nc.dram_tensor("W_hh0", [H, H], F32, kind="ExternalInput")
    b0_d = nc.dram_tensor("b_h0", [H], F32, kind="ExternalInput")
    wxh1_d = nc.dram_tensor("W_xh1", [H, H], F32, kind="ExternalInput")
    whh1_d = nc.dram_tensor("W_hh1", [H, H], F32, kind="ExternalInput")
    b1_d = nc.dram_tensor("b_h1", [H], F32, kind="ExternalInput")
    ident_d = nc.dram_tensor("ident", [128, 128], BF16, kind="ExternalInput")
    out_d = nc.dram_tensor("out", [B, T, H], F32, kind="ExternalOutput")

    with tile.TileContext(nc) as tc:
        _body(tc, nch, x_d, (wxh0_d, whh0_d, b0_d, wxh1_d, whh1_d, b1_d),
              ident_d, out_d, reps=reps)
    nc.compile()
    return nc


def _body(tc, nch, x_d, w_d, ident_d, out_d, reps=1):
    import contextlib

    nc = tc.nc
    wxh0_d, whh0_d, b0_d, wxh1_d, whh1_d, b1_d = w_d

    ctx = contextlib.ExitStack()
    with ctx:
        consts = ctx.enter_context(tc.tile_pool(name="consts", bufs=1))
        wstage = ctx.enter_context(tc.tile_pool(name="wstage", bufs=2))
        xpool = ctx.enter_context(tc.tile_pool(name="xpool", bufs=2))
        state = ctx.enter_context(tc.tile_pool(name="state", bufs=1))
        stg = ctx.enter_context(tc.tile_pool(name="stg", bufs=3))
        ps_l0 = ctx.enter_context(tc.tile_pool(name="ps_l0", bufs=2, space="PSUM"))
        ps_l1 = ctx.enter_context(tc.tile_pool(name="ps_l1", bufs=2, space="PSUM"))
        ps_xt = ctx.enter_context(tc.tile_pool(name="ps_xt", bufs=2, space="PSUM"))
        ps_ot = ctx.enter_context(tc.tile_pool(name="ps_ot", bufs=2, space="PSUM"))

        # ---- one-time constants: identity, ones, weights (fp32 -> bf16) ----
        ident = consts.tile([128, 128], BF16, tag="ident")
        nc.sync.dma_start(ident[:], ident_d.ap()[:])
        ones = consts.tile([1, 128], BF16, tag="ones")
        nc.gpsimd.memset(ones[:], 1.0)

        def load_w(dram_ap, rows, name):
            # one [128,128] tile per (kc, mc) chunk so every matmul's
            # stationary operand is a whole tile at offset 0 -- keeps the
            # compiler's fast-weight-load eligibility unambiguous
            tiles = []
            for kc in range(rows // 128):
                tmp = wstage.tile([128, H], F32, tag="wtmp")
                nc.sync.dma_start(tmp[:], dram_ap[kc * 128:(kc + 1) * 128, :])
                row = []
                for mc in range(4):
                    wt = consts.tile([128, 128], BF16, tag=f"{name}_{kc}_{mc}")
                    nc.vector.tensor_copy(wt[:], tmp[:, mc * 128:(mc + 1) * 128])
                    row.append(wt)
                tiles.append(row)
            return tiles

        wxh0 = load_w(wxh0_d.ap(), I, "wxh0")   # 2 tiles [128, 512]
        whh0 = load_w(whh0_d.ap(), H, "whh0")   # 4 tiles
        wxh1 = load_w(wxh1_d.ap(), H, "wxh1")
        whh1 = load_w(whh1_d.ap(), H, "whh1")

        def load_b(dram_ap, name):
            tmp = wstage.tile([1, H], F32, tag="btmp")
            nc.sync.dma_start(tmp[:], dram_ap.unsqueeze(0))
            bt = consts.tile([1, H], BF16, tag=name)
            nc.vector.tensor_copy(bt[:], tmp[:])
            return bt

        b0 = load_b(b0_d.ap(), "b0")
        b1 = load_b(b1_d.ap(), "b1")

        # ---- recurrent state rings: 32 slots of [128, 32] (free = hc*8+b) ----
        h0T = state.tile([128, 32 * 32], BF16, tag="h0T")
        h1T = state.tile([128, 32 * 32], BF16, tag="h1T")
        nc.gpsimd.memset(h0T[:, 31 * 32:32 * 32], 0.0)  # h_{-1} = 0
        nc.gpsimd.memset(h1T[:, 31 * 32:32 * 32], 0.0)

        x_src = x_d.ap().rearrange("b (c t) i -> c t b i", t=CH)       # [64,16,8,256]
        out_dst = out_d.ap().rearrange(
            "b (c g t) (hc p) -> c g t hc b p", g=CH // 4, t=4, p=128)

        def l0_chunk(c):
            # x load -> bf16 -> PE-transpose -> xT [128(i), 2ic*128(t,b)]
            xf = xpool.tile([128, I], F32, tag="xf")
            nc.sync.dma_start(xf[:], x_src[c])
            xb = xpool.tile([128, I], BF16, tag="xb")
            nc.vector.tensor_copy(xb[:], xf[:])
            xtp = ps_xt.tile([128, I], BF16, tag="xtp")
            for ic in range(2):
                # start=True zeroes the whole 2KB PSUM bank region, so only
                # the first write into a fresh bank may set it.
                nc.tensor.matmul(xtp[:, ic * 128:(ic + 1) * 128],
                                 xb[:, ic * 128:(ic + 1) * 128], ident[:],
                                 is_transpose=True, start=(ic == 0),
                                 stop=(ic == 1), skip_group_check=True)
            xT = xpool.tile([128, I], BF16, tag="xT")
            nc.vector.tensor_copy(xT[:], xtp[:])

            # PSUM bank layout: free = mc*128 + tt*8 + b  (mc-major, so every
            # matmul out is a contiguous 2D slice)
            ps = ps_l0.tile([128, 512], F32, tag="ps0")
            psv = ps[:].rearrange("p (m t b) -> p t m b", m=4, t=CH, b=B)
            # bias + batched input projection (transposed), both into PSUM
            for hc in range(4):
                _mm(nc, ps[:, hc * 128:(hc + 1) * 128],
                    b0[:, hc * 128:(hc + 1) * 128], ones[:],
                    start=(hc == 0), stop=False)
            for hc in range(4):
                for ic in range(2):
                    _mm(nc, ps[:, hc * 128:(hc + 1) * 128],
                        wxh0[ic][hc][:],
                        xT[:, ic * 128:(ic + 1) * 128],
                        start=False, stop=False)
            # recurrence
            for tt in range(CH):
                slot = (c % 2) * CH + tt
                prev = ((slot - 1) % 32) * 32
                for mc in range(4):
                    for kc in range(4):
                        o = mc * 128 + tt * 8
                        _mm(nc, ps[:, o: o + 8],
                            whh0[kc][mc][:],
                            h0T[:, prev + kc * 8: prev + kc * 8 + 8],
                            start=False, stop=(kc == 3))
                nc.scalar.activation(
                    h0T[:, slot * 32:(slot + 1) * 32].rearrange(
                        "p (m b) -> p m b", b=B),
                    psv[:, tt], Tanh)

        def l1_chunk(c):
            base = (c % 2) * CH
            ps = ps_l1.tile([128, 512], F32, tag="ps1")
            psv = ps[:].rearrange("p (m t b) -> p t m b", m=4, t=CH, b=B)
            h0Tv = h0T[:].rearrange("p (s m b) -> p s m b", s=32, b=B)
            for hc in range(4):
                _mm(nc, ps[:, hc * 128:(hc + 1) * 128],
                    b1[:, hc * 128:(hc + 1) * 128], ones[:],
                    start=(hc == 0), stop=False)
            for hc in range(4):
                for kc in range(4):
                    _mm(nc, ps[:, hc * 128:(hc + 1) * 128],
                        wxh1[kc][hc][:],
                        h0Tv[:, base:base + CH, kc], start=False, stop=False)
            for tt in range(CH):
                slot = base + tt
                prev = ((slot - 1) % 32) * 32
                for mc in range(4):
                    for kc in range(4):
                        o = mc * 128 + tt * 8
                        _mm(nc, ps[:, o: o + 8],
                            whh1[kc][mc][:],
                            h1T[:, prev + kc * 8: prev + kc * 8 + 8],
                            start=False, stop=(kc == 3))
                nc.scalar.activation(
                    h1T[:, slot * 32:(slot + 1) * 32].rearrange(
                        "p (m b) -> p m b", b=B),
                    psv[:, tt], Tanh)
                if tt % 4 == 3:
                    g = tt // 4
                    tp = ps_ot.tile([128, 128], BF16, tag="otp")
                    nc.tensor.transpose(
                        tp[:], h1T[:, (slot - 3) * 32:(slot + 1) * 32], ident[:])
                    st = stg.tile([128, 128], F32, tag="ost")
                    nc.vector.tensor_copy(st[:], tp[:])
                    nc.sync.dma_start(out_dst[c, g], st[:])

        def main_loop():
            for c in range(nch + 1):
                if c < nch:
                    l0_chunk(c)
                if c >= 1:
                    l1_chunk(c - 1)

        if reps > 1:
            # timing mode: repeat the whole body on-device so the kernel time
            # dominates the (network-tunneled) host<->device transfer wall.
            with tc.For_i(0, reps, 1):
                main_loop()
        else:
            main_loop()


_NC_CACHE = {}


def _get_nc(nch=NCH):
    if nch not in _NC_CACHE:
        _NC_CACHE[nch] = build_nc(nch)
    return _NC_CACHE[nch]


def kernel(**inputs):
    x = np.asarray(inputs["inputs"], dtype=np.float32)
    ident = np.eye(128, dtype=ml_dtypes.bfloat16)
    shared = {
        "W_xh0": np.asarray(inputs["W_xh0"], np.float32),
        "W_hh0": np.asarray(inputs["W_hh0"], np.float32),
        "b_h0": np.asarray(inputs["b_h0"], np.float32),
        "W_xh1": np.asarray(inputs["W_xh1"], np.float32),
        "W_hh1": np.asarray(inputs["W_hh1"], np.float32),
        "b_h1": np.asarray(inputs["b_h1"], np.float32),
        "ident": ident,
    }
    in_maps = [dict(shared, x=np.ascontiguousarray(x[c * B:(c + 1) * B]))
               for c in range(NCORES)]
    nc = _get_nc()
    res = run_bass_kernel_spmd(nc, in_maps, core_ids=list(range(NCORES)))
    return np.concatenate([r["out"] for r in res.results], axis=0)

